# revision 28
# baseline (speedup 1.0000x reference)
"""Trainium2 Bass kernel for ChebConv with spatial attention.

Reference computation (per sample b):
    A_k = cheb[k] * att[b]                    (elementwise, [N,N])
    rhs_k = A_k @ x[b,t]                      ([N,N] @ [N,F_IN] for all t)
    out[b,t] = relu(sum_k rhs_k @ Theta[k])   ([N,F_OUT])

Sharding: data-parallel over batch B=8, one sample per NeuronCore.
cheb/Theta replicated. Host prep passes transposed adjacency factors
(attT uint8, chebT int8) so the on-chip elementwise product directly
yields A^T tiles, which the PE consumes as the moving matmul operand
with contraction over j on the partition dim — no on-chip transposes.

Quantization: att rides as uint8 (U[0,1) fixed point); cheb rides as
int8 with PER-J scales s[j] = max_{k,i}|cheb[k,i,j]|/127; both the
1/255 and s[j] rescales fold into xT rows on the host (RT contracts
over j, so a per-j factor on x is exact; thetaM stays unscaled).
Halving cheb (the 6 MB dominant stream) drops the k0 DMA requirement
from ~300 GB/s to ~220 GB/s. Measured rel err ~9.6e-3 vs the 2e-2
gate (matches the CPU simulation of the exact pipeline).

Per-core dataflow:
  phase B: per (k, j): AT = att_u8 * cheb_i8 (DVE, bf16 out), then 8
           matmuls RT[(t,f)=128, i=512] += X_j^T @ AT into 8 PSUM
           chains (one per tg/ih), j-accumulated. After each k, the
           PSUM->SBUF bf16 casts run on DVE+ACT interleaved in the
           order the next k consumes the banks (0,2,4,6,1,3,5,7) so
           the k-boundary wait is ~0.9us. Next k's j0/j1 cheb is
           prefetched mid-k on the SWDGE (gpsimd) queue — pool-tag
           gating naturally delays its issue past the HWDGE spin-up
           window — with products on DVE at j==6.
  phase C: out[i=128, (t,o)] += RT^T @ thetaM_k per tg in order
           0,1(psA) 2,3(psB); thetaM zero-pads Theta[k] per 32-row
           strip so one N=256 matmul covers a whole t-group (moving-
           col floor 24576 cycles; (k,f)-packed variants lose to the
           32-partition cast penalty). relu(psA) on ACT, relu(psB) on
           DVE; final block quarter-split so the last store is small.

DMA schedule: per-j 1-wide transfers on the two HWDGE rings (per-queue
~100-120 GB/s regardless of transfer size; 2-wide grouping and SWDGE
bulk routing both measured slower): per j, att_j+x_j on one queue,
cheb_j on the other, alternating; j0 is split in halves with x0 on
sync slot 2 — its receipt (~10.7-11.9us; receipts, not data arrival,
gate consumers) is the first-matmul gate. k1/k2 cheb re-uses the
then-idle HWDGE rings; theta rides gpsimd after the cbp prefetch.
WARMUP=8 512-col matmuls on a zero tile bridge PE-queue release
(~8.3us) to the first real matmul with no idle gap: the HAM full-
clock release triggers ~2-3us after sustained 512-col activity
begins, so real matmuls run at 2.4 GHz immediately (an idle gap
resets the ramp and costs ~2us of throttled matmuls).

Timeline at full clock (exec ~72.3-73.5us; NTFF exec excludes ~5.9us
of preamble): body starts 7.15, first real matmul ~12.0, phase B ends
~56.5 (41.3us PE floor + ~1.5us k0 receipt jitter + 2x ~0.9us
k-boundaries), phase C ends ~67.5 (10.2us floor), last store receipt
~70.5, fixed TileContext teardown (DRAIN token ring across 5 engines,
runs at post-idle half clock; PE keep-alive tricks just delay it) to
~79.5. Device P-state varies run to run (2.0 vs 2.4 GHz classes,
+-15%); compare configs best-of-8 within one window.
"""

import numpy as np
from contextlib import ExitStack

B, T, N, F_IN, F_OUT, K = 8, 16, 1024, 32, 64, 3
NJ = N // 128  # j tiles (contraction)
NI = N // 128  # i tiles (output rows)
NTG = 4        # t-groups of 4 t's -> 128 = 4*32 partitions
TF = T * F_IN   # 512
TO = T * F_OUT  # 1024
WARMUP = 6

_LAST_RESULTS = None  # BassKernelResults of the most recent run (for test harness)


def _build_bass():
    import concourse.mybir as mybir
    import concourse.tile as tile
    from concourse import bacc
    from concourse.bass import ts

    f32 = mybir.dt.float32
    bf16 = mybir.dt.bfloat16
    u8 = mybir.dt.uint8
    i8 = mybir.dt.int8
    nc = bacc.Bacc()

    xT_d = nc.dram_tensor("xT", [N, TF], bf16, kind="ExternalInput")
    attT_d = nc.dram_tensor("attT", [N, N], u8, kind="ExternalInput")
    chebT_d = nc.dram_tensor("chebT", [K * N, N], i8, kind="ExternalInput")
    th_d = nc.dram_tensor("thetaM", [128, K * 4 * F_OUT], bf16, kind="ExternalInput")
    out_d = nc.dram_tensor("out", [N, TO], bf16, kind="ExternalOutput")

    with tile.TileContext(nc) as tc, ExitStack() as ctx:
        x_pool = ctx.enter_context(tc.tile_pool(name="x", bufs=1))
        att_pool = ctx.enter_context(tc.tile_pool(name="att", bufs=1))
        cheb_pool = ctx.enter_context(tc.tile_pool(name="cheb", bufs=8))
        at_pool = ctx.enter_context(tc.tile_pool(name="at", bufs=6))
        rt_pool = ctx.enter_context(tc.tile_pool(name="rt", bufs=K * NTG))
        th_pool = ctx.enter_context(tc.tile_pool(name="th", bufs=1))
        ob_pool = ctx.enter_context(tc.tile_pool(name="ob", bufs=3))
        wz_pool = ctx.enter_context(tc.tile_pool(name="wz", bufs=1))

        q = [nc.sync, nc.scalar]

        xg, attg = [None] * NJ, [None] * NJ

        rts = [[None] * NTG for _ in range(K)]
        th = None
        with tc.tile_pool(name="psumB", bufs=1, space="PSUM") as pb:
            # PE warm-up on a dedicated zero tile. 512-col matmuls (same
            # shape as the real ones) trigger the HAM full-clock release
            # ~2-3us after they start, so the clock is already full when
            # the first real matmul data lands ~10.7us.
            wz = wz_pool.tile([128, 512], bf16, name="warmz")
            nc.vector.memset(wz[:], 0)
            wps = pb.tile([128, 512], f32, name="warmps", tag="chain7")
            for _ in range(WARMUP):
                nc.tensor.matmul(wps[:], wz[:, 0:128], wz[:], start=True, stop=True)
            for _ in range(3):
                # short tail matmuls: keep HAM duty continuous but free the
                # PE quickly once the first real matmul data lands
                nc.tensor.matmul(wps[:, 0:128], wz[:, 0:128], wz[:, 0:128],
                                 start=True, stop=True)

            at_pre = {}
            for k in range(K):
                chains = [
                    pb.tile([128, 512], f32, name=f"chain{k}_{c}", tag=f"chain{c}")
                    for c in range(2 * NTG)
                ]
                for j in range(NJ):
                    first = k == 0 and j == 0
                    pre = k > 0 and j < 2
                    if k == 0:
                        a = att_pool.tile([128, N], u8,
                                          name=f"att{j}", tag=f"att{j}")
                        xt = x_pool.tile([128, TF], bf16,
                                         name=f"x{j}", tag=f"x{j}")
                        attg[j] = a
                        xg[j] = xt
                    if not pre:
                        cb = cheb_pool.tile([128, N], i8, name=f"cb{k}_{j}",
                                            tag="cb")
                    if k == 0:
                        if first:
                            # slot-1 halves on both queues feed the first
                            # product; x0 whole on sync slot 2 is the
                            # first-matmul gate (receipt ~10.7us)
                            nc.sync.dma_start(a[:, 0:512], attT_d[0:128, 0:512])
                            nc.scalar.dma_start(cb[:, 0:512],
                                                chebT_d[0:128, 0:512])
                            nc.sync.dma_start(xt[:], xT_d[0:128, :])
                            nc.scalar.dma_start(a[:, 512:1024],
                                                attT_d[0:128, 512:1024])
                            nc.sync.dma_start(cb[:, 512:1024],
                                              chebT_d[0:128, 512:1024])
                        else:
                            # att_j+x_j on one queue, cheb_j on the other,
                            # alternating: balanced 384KB per j-pair side
                            qa, qc = q[j % 2], q[(j + 1) % 2]
                            qa.dma_start(a[:], attT_d[ts(j, 128), :])
                            qc.dma_start(cb[:], chebT_d[ts(j, 128), :])
                            qa.dma_start(xt[:], xT_d[ts(j, 128), :])
                    elif not pre:
                        # k1/k2 cheb rides sync ONLY: the scalar (ACT)
                        # engine must not queue DMA-issue instructions
                        # ahead of its k-boundary PSUM casts
                        r0 = k * N + j * 128
                        nc.sync.dma_start(cb[:], chebT_d[r0:r0 + 128, :])
                    if pre:
                        at = at_pre.pop((k, j))
                    else:
                        at = at_pool.tile([128, N], bf16, name=f"at{k}_{j}",
                                          tag="at")
                        if first:
                            nc.vector.tensor_mul(at[:, 0:512],
                                                 attg[j][:, 0:512],
                                                 cb[:, 0:512])
                            nc.vector.tensor_mul(at[:, 512:1024],
                                                 attg[j][:, 512:1024],
                                                 cb[:, 512:1024])
                        else:
                            nc.vector.tensor_mul(at[:], attg[j][:], cb[:])
                    for ih in range(2):
                        for tg in range(NTG):
                            nc.tensor.matmul(
                                chains[tg * 2 + ih][:],
                                xg[j][:, ts(tg, 128)],
                                at[:, ts(ih, 512)],
                                start=(j == 0),
                                stop=(j == NJ - 1),
                            )
                    if j == 6 and k < K - 1:
                        # prefetch next k's j0/j1 cheb on SWDGE (gpsimd) —
                        # products run mid-k on DVE so the k-boundary only
                        # waits on the chain casts
                        kn = k + 1
                        for jp in range(2):
                            cbp = cheb_pool.tile([128, N], i8,
                                                 name=f"cbp{kn}_{jp}", tag="cb")
                            r0 = kn * N + jp * 128
                            nc.gpsimd.dma_start(cbp[:], chebT_d[r0:r0 + 128, :])
                            atp = at_pool.tile([128, N], bf16,
                                               name=f"atp{kn}_{jp}", tag="at")
                            nc.vector.tensor_mul(atp[:], attg[jp][:], cbp[:])
                            at_pre[(kn, jp)] = atp
                        if k == 0:
                            # theta (phase C only): on gpsimd AFTER the cbp
                            # prefetches so it stays clear of the x stream
                            th = th_pool.tile([128, K * 4 * F_OUT], bf16)
                            nc.gpsimd.dma_start(th[:], th_d[:, :])
                # PSUM->SBUF bf16 casts on DVE+ACT (gpsimd cannot read
                # PSUM), interleaved in the order the next k's matmuls
                # consume the banks (chains 0,2,4,6 then 1,3,5,7) so the
                # PE restart is paced by pairs of banks, not one engine
                # crawling through the even banks.
                rtk = [rt_pool.tile([128, N], bf16, name=f"rt{k}_{tg}",
                                    tag="rt")
                       for tg in range(NTG)]
                order = [0, 2, 4, 6, 1, 3, 5, 7]
                for n, c in enumerate(order):
                    tg, ih = c // 2, c % 2
                    dst = rtk[tg][:, ts(ih, 512)]
                    if n % 2 == 1:
                        nc.scalar.copy(dst, chains[c][:])
                    else:
                        nc.vector.tensor_copy(dst, chains[c][:])
                for tg in range(NTG):
                    rts[k][tg] = rtk[tg]

            # phase C: out[i-block, (t,o)] = relu(sum_k RT_k^T @ thetaM_k).
            # One matmul per (tg, k): full K=128 contraction where thetaM
            # zero-pads Theta[k] per 32-row strip, producing the 4 t's of
            # the t-group in one N=256 matmul. tg order 0,1 (psA) then
            # 2,3 (psB): psA is complete at 50% of the i-block so its
            # relu (ACT) and half-block store overlap the psB matmuls;
            # relu(psB) runs on DVE. PSUM tiles reuse the chain{c} tags
            # so each bank frees as soon as its k=2 chain is cast.
            ob = None
            for ib in range(NI):
                psA = pb.tile([128, 512], f32, name=f"psA{ib}",
                              tag=f"chain{(2 * ib) % 8}")
                psB = pb.tile([128, 512], f32, name=f"psB{ib}",
                              tag=f"chain{(2 * ib + 1) % 8}")
                for tg, ps in ((0, psA), (1, psA), (2, psB), (3, psB)):
                    for k in range(K):
                        nc.tensor.matmul(
                            ps[:, ts(tg % 2, 4 * F_OUT)],
                            rts[k][tg][:, ts(ib, 128)],
                            th[:, ts(k, 4 * F_OUT)],
                            start=(k == 0),
                            stop=(k == K - 1),
                        )
                ob = ob_pool.tile([128, TO], bf16)
                nc.scalar.activation(ob[:, 0:512], psA[:],
                                     mybir.ActivationFunctionType.Relu)
                nc.sync.dma_start(out_d[ts(ib, 128), 0:512], ob[:, 0:512])
                if ib < NI - 1:
                    nc.vector.tensor_scalar_max(ob[:, 512:1024], psB[:], 0.0)
                    nc.scalar.dma_start(out_d[ts(ib, 128), 512:1024],
                                        ob[:, 512:1024])
                else:
                    # quarter-split the final block so the very last store is
                    # small and issues right after a short relu
                    nc.vector.tensor_scalar_max(ob[:, 512:768], psB[:, 0:256],
                                                0.0)
                    nc.scalar.dma_start(out_d[ts(ib, 128), 512:768],
                                        ob[:, 512:768])
                    nc.vector.tensor_scalar_max(ob[:, 768:1024], psB[:, 256:512],
                                                0.0)
                    nc.sync.dma_start(out_d[ts(ib, 128), 768:1024],
                                      ob[:, 768:1024])



    nc.compile()
    return nc


def _prep_inputs(x, att, cheb, Theta):
    import ml_dtypes

    bf16 = ml_dtypes.bfloat16
    # per-j int8 scales for cheb; s[j] and the att 1/255 fold into xT rows
    # (exact: RT contracts over j, x carries any per-j factor)
    s = np.abs(cheb).max(axis=(0, 1)) / 127.0          # [N] over (k, i)
    chebT = np.clip(np.rint(cheb / s[None, None, :]), -127, 127).astype(np.int8)
    chebT = np.ascontiguousarray(chebT.transpose(0, 2, 1)).reshape(K * N, N)
    xscale = (s * (1.0 / 255.0)).astype(np.float32)    # [N] per-j factor

    # zero-padded Theta: strip tt of the partition dim carries Theta[k]
    # only in the tt-th 64-col block of k's 256-col group
    thetaM = np.zeros((128, K * 4 * F_OUT), np.float32)
    for tt in range(4):
        for k in range(K):
            thetaM[tt * 32:(tt + 1) * 32,
                   k * 4 * F_OUT + tt * F_OUT:
                   k * 4 * F_OUT + (tt + 1) * F_OUT] = Theta[k]
    thetaM = thetaM.astype(bf16)

    in_maps = []
    for b in range(B):
        xb = np.ascontiguousarray(x[b].transpose(1, 0, 2)).reshape(N, TF)
        in_maps.append({
            "xT": (xb * xscale[:, None]).astype(bf16),
            "attT": np.rint(np.ascontiguousarray(att[b].T) * 255.0
                            ).astype(np.uint8),
            "chebT": chebT,
            "thetaM": thetaM,
        })
    return in_maps


def kernel(**inputs: np.ndarray) -> np.ndarray:
    global _LAST_RESULTS
    from concourse.bass_utils import run_bass_kernel_spmd

    x = np.asarray(inputs["x"], dtype=np.float32)
    att = np.asarray(inputs["spatial_attention"], dtype=np.float32)
    cheb = np.asarray(inputs["cheb"], dtype=np.float32)
    Theta = np.asarray(inputs["Theta"], dtype=np.float32)

    in_maps = _prep_inputs(x, att, cheb, Theta)
    nc = _build_bass()
    res = run_bass_kernel_spmd(nc, in_maps, core_ids=list(range(B)))
    _LAST_RESULTS = res

    out = np.stack(
        [r["out"].astype(np.float32).reshape(N, T, F_OUT).transpose(1, 0, 2)
         for r in res.results]
    )
    return out


# revision 29
# speedup vs baseline: 1.1308x; 1.1308x over previous
"""Trainium2 Bass kernel for ChebConv with spatial attention.

Reference computation (per sample b):
    A_k = cheb[k] * att[b]                    (elementwise, [N,N])
    rhs_k = A_k @ x[b,t]                      ([N,N] @ [N,F_IN] for all t)
    out[b,t] = relu(sum_k rhs_k @ Theta[k])   ([N,F_OUT])

Sharding: data-parallel over batch B=8, one sample per NeuronCore.
cheb/Theta replicated. Host prep passes transposed adjacency factors
(attT uint8, chebT int8) so the on-chip elementwise product directly
yields A^T tiles, which the PE consumes as the moving matmul operand
with contraction over j on the partition dim — no on-chip transposes.

Quantization: att rides as uint8 (U[0,1) fixed point); cheb rides as
int8 with PER-J scales s[j] = max_{k,i}|cheb[k,i,j]|/127; both the
1/255 and s[j] rescales fold into xT rows on the host (RT contracts
over j, so a per-j factor on x is exact; thetaM stays unscaled).
Halving cheb (the 6 MB dominant stream) drops the k0 DMA requirement
from ~300 GB/s to ~220 GB/s. Measured rel err ~9.6e-3 vs the 2e-2
gate (matches the CPU simulation of the exact pipeline).

Per-core dataflow:
  phase B: per (k, j): AT = att_u8 * cheb_i8 (DVE, bf16 out), then 8
           matmuls RT[(t,f)=128, i=512] += X_j^T @ AT into 8 PSUM
           chains (one per tg/ih), j-accumulated. After each k, the
           PSUM->SBUF bf16 casts run on DVE+ACT interleaved in the
           order the next k consumes the banks (0,2,4,6,1,3,5,7) so
           the k-boundary wait is ~0.9us. Next k's j0/j1 cheb is
           prefetched mid-k on the SWDGE (gpsimd) queue — pool-tag
           gating naturally delays its issue past the HWDGE spin-up
           window — with products on DVE at j==6.
  phase C: out[i=128, (t,o)] += RT^T @ thetaM_k per tg in order
           0,1(psA) 2,3(psB); thetaM zero-pads Theta[k] per 32-row
           strip so one N=256 matmul covers a whole t-group (moving-
           col floor 24576 cycles; (k,f)-packed variants lose to the
           32-partition cast penalty). relu(psA) on ACT, relu(psB) on
           DVE; final block quarter-split so the last store is small.

DMA schedule: per-j 1-wide transfers on the two HWDGE rings (per-queue
~100-120 GB/s regardless of transfer size; 2-wide grouping and SWDGE
bulk routing both measured slower): per j, att_j+x_j on one queue,
cheb_j on the other, alternating; j0 is split in halves with x0 on
sync slot 2 — its receipt (~10.7-11.9us; receipts, not data arrival,
gate consumers) is the first-matmul gate. k1/k2 cheb re-uses the
then-idle HWDGE rings; theta rides gpsimd after the cbp prefetch.
WARMUP=6 512-col matmuls plus 3 short 128-col tail matmuls bridge
PE-queue release (~8.3us) to the first real matmul with no idle gap:
the HAM full-clock release triggers ~2-3us after sustained 512-col
activity begins, so real matmuls run at full clock immediately (an
idle gap resets the ramp and costs ~2us of throttled matmuls); the
short tail frees the PE within ~130ns of the x0 receipt landing.

Timeline at full clock (NTFF exec excludes ~5.9us of preamble):
body starts 7.15, first real matmul ~11.5-12.0 (x0 receipt-gated),
then a gapless PE stream (total measured PE gaps ~0.4us): phase B
41.3us floor, phase C 10.2us floor, last store receipt ~+3, fixed
TileContext teardown (DRAIN token ring across 5 engines, runs at
post-idle half clock; PE keep-alive tricks just delay it) ~+9.
Device P-state varies run to run (2.0 vs 2.4 GHz classes, +-15%);
compare configs best-of-8 within one window; 2.4GHz-class exec
~69.5-70.5us.
"""

import numpy as np
from contextlib import ExitStack

B, T, N, F_IN, F_OUT, K = 8, 16, 1024, 32, 64, 3
NJ = N // 128  # j tiles (contraction)
NI = N // 128  # i tiles (output rows)
NTG = 4        # t-groups of 4 t's -> 128 = 4*32 partitions
TF = T * F_IN   # 512
TO = T * F_OUT  # 1024
WARMUP = 6

_LAST_RESULTS = None  # BassKernelResults of the most recent run (for test harness)


def _build_bass():
    import concourse.mybir as mybir
    import concourse.tile as tile
    from concourse import bacc
    from concourse.bass import ts

    f32 = mybir.dt.float32
    bf16 = mybir.dt.bfloat16
    u8 = mybir.dt.uint8
    i8 = mybir.dt.int8
    nc = bacc.Bacc()

    xT_d = nc.dram_tensor("xT", [N, TF], bf16, kind="ExternalInput")
    attT_d = nc.dram_tensor("attT", [N, N], u8, kind="ExternalInput")
    chebT_d = nc.dram_tensor("chebT", [K * N, N], i8, kind="ExternalInput")
    th_d = nc.dram_tensor("thetaM", [128, K * 4 * F_OUT], bf16, kind="ExternalInput")
    out_d = nc.dram_tensor("out", [N, TO], bf16, kind="ExternalOutput")

    with tile.TileContext(nc) as tc, ExitStack() as ctx:
        x_pool = ctx.enter_context(tc.tile_pool(name="x", bufs=1))
        att_pool = ctx.enter_context(tc.tile_pool(name="att", bufs=1))
        cheb_pool = ctx.enter_context(tc.tile_pool(name="cheb", bufs=8))
        at_pool = ctx.enter_context(tc.tile_pool(name="at", bufs=6))
        rt_pool = ctx.enter_context(tc.tile_pool(name="rt", bufs=K * NTG))
        th_pool = ctx.enter_context(tc.tile_pool(name="th", bufs=1))
        ob_pool = ctx.enter_context(tc.tile_pool(name="ob", bufs=3))
        wz_pool = ctx.enter_context(tc.tile_pool(name="wz", bufs=1))

        q = [nc.sync, nc.scalar]

        xg, attg = [None] * NJ, [None] * NJ

        rts = [[None] * NTG for _ in range(K)]
        th = None
        with tc.tile_pool(name="psumB", bufs=1, space="PSUM") as pb:
            # PE warm-up on a dedicated zero tile. 512-col matmuls (same
            # shape as the real ones) trigger the HAM full-clock release
            # ~2-3us after they start, so the clock is already full when
            # the first real matmul data lands ~10.7us.
            wz = wz_pool.tile([128, 512], bf16, name="warmz")
            nc.vector.memset(wz[:], 0)
            wps = pb.tile([128, 512], f32, name="warmps", tag="chain7")
            for _ in range(WARMUP):
                nc.tensor.matmul(wps[:], wz[:, 0:128], wz[:], start=True, stop=True)
            for _ in range(3):
                # short tail matmuls: keep HAM duty continuous but free the
                # PE quickly once the first real matmul data lands
                nc.tensor.matmul(wps[:, 0:128], wz[:, 0:128], wz[:, 0:128],
                                 start=True, stop=True)

            at_pre = {}
            for k in range(K):
                chains = [
                    pb.tile([128, 512], f32, name=f"chain{k}_{c}", tag=f"chain{c}")
                    for c in range(2 * NTG)
                ]
                for j in range(NJ):
                    first = k == 0 and j == 0
                    pre = k > 0 and j < 2
                    if k == 0:
                        a = att_pool.tile([128, N], u8,
                                          name=f"att{j}", tag=f"att{j}")
                        xt = x_pool.tile([128, TF], bf16,
                                         name=f"x{j}", tag=f"x{j}")
                        attg[j] = a
                        xg[j] = xt
                    if not pre:
                        cb = cheb_pool.tile([128, N], i8, name=f"cb{k}_{j}",
                                            tag="cb")
                    if k == 0:
                        if first:
                            # slot-1 halves on both queues feed the first
                            # product; x0 whole on sync slot 2 is the
                            # first-matmul gate (receipt ~10.7us)
                            nc.sync.dma_start(a[:, 0:512], attT_d[0:128, 0:512])
                            nc.scalar.dma_start(cb[:, 0:512],
                                                chebT_d[0:128, 0:512])
                            nc.sync.dma_start(xt[:], xT_d[0:128, :])
                            nc.scalar.dma_start(a[:, 512:1024],
                                                attT_d[0:128, 512:1024])
                            nc.sync.dma_start(cb[:, 512:1024],
                                              chebT_d[0:128, 512:1024])
                        else:
                            # att_j+x_j on one queue, cheb_j on the other,
                            # alternating: balanced 384KB per j-pair side
                            qa, qc = q[j % 2], q[(j + 1) % 2]
                            qa.dma_start(a[:], attT_d[ts(j, 128), :])
                            qc.dma_start(cb[:], chebT_d[ts(j, 128), :])
                            qa.dma_start(xt[:], xT_d[ts(j, 128), :])
                    elif not pre:
                        # k1/k2 cheb rides sync ONLY: the scalar (ACT)
                        # engine must not queue DMA-issue instructions
                        # ahead of its k-boundary PSUM casts
                        r0 = k * N + j * 128
                        nc.sync.dma_start(cb[:], chebT_d[r0:r0 + 128, :])
                    if pre:
                        at = at_pre.pop((k, j))
                    else:
                        at = at_pool.tile([128, N], bf16, name=f"at{k}_{j}",
                                          tag="at")
                        if first:
                            nc.vector.tensor_mul(at[:, 0:512],
                                                 attg[j][:, 0:512],
                                                 cb[:, 0:512])
                            nc.vector.tensor_mul(at[:, 512:1024],
                                                 attg[j][:, 512:1024],
                                                 cb[:, 512:1024])
                        else:
                            nc.vector.tensor_mul(at[:], attg[j][:], cb[:])
                    for ih in range(2):
                        for tg in range(NTG):
                            nc.tensor.matmul(
                                chains[tg * 2 + ih][:],
                                xg[j][:, ts(tg, 128)],
                                at[:, ts(ih, 512)],
                                start=(j == 0),
                                stop=(j == NJ - 1),
                            )
                    if j == 6 and k < K - 1:
                        # prefetch next k's j0/j1 cheb on SWDGE (gpsimd) —
                        # products run mid-k on DVE so the k-boundary only
                        # waits on the chain casts
                        kn = k + 1
                        for jp in range(2):
                            cbp = cheb_pool.tile([128, N], i8,
                                                 name=f"cbp{kn}_{jp}", tag="cb")
                            r0 = kn * N + jp * 128
                            nc.gpsimd.dma_start(cbp[:], chebT_d[r0:r0 + 128, :])
                            atp = at_pool.tile([128, N], bf16,
                                               name=f"atp{kn}_{jp}", tag="at")
                            nc.vector.tensor_mul(atp[:], attg[jp][:], cbp[:])
                            at_pre[(kn, jp)] = atp
                        if k == 0:
                            # theta (phase C only): on gpsimd AFTER the cbp
                            # prefetches so it stays clear of the x stream
                            th = th_pool.tile([128, K * 4 * F_OUT], bf16)
                            nc.gpsimd.dma_start(th[:], th_d[:, :])
                # PSUM->SBUF bf16 casts on DVE+ACT (gpsimd cannot read
                # PSUM), interleaved in the order the next k's matmuls
                # consume the banks (chains 0,2,4,6 then 1,3,5,7) so the
                # PE restart is paced by pairs of banks, not one engine
                # crawling through the even banks.
                rtk = [rt_pool.tile([128, N], bf16, name=f"rt{k}_{tg}",
                                    tag="rt")
                       for tg in range(NTG)]
                order = [0, 2, 4, 6, 1, 3, 5, 7]
                for n, c in enumerate(order):
                    tg, ih = c // 2, c % 2
                    dst = rtk[tg][:, ts(ih, 512)]
                    if n % 2 == 1:
                        nc.scalar.copy(dst, chains[c][:])
                    else:
                        nc.vector.tensor_copy(dst, chains[c][:])
                for tg in range(NTG):
                    rts[k][tg] = rtk[tg]

            # phase C: out[i-block, (t,o)] = relu(sum_k RT_k^T @ thetaM_k).
            # One matmul per (tg, k): full K=128 contraction where thetaM
            # zero-pads Theta[k] per 32-row strip, producing the 4 t's of
            # the t-group in one N=256 matmul. tg order 0,1 (psA) then
            # 2,3 (psB): psA is complete at 50% of the i-block so its
            # relu (ACT) and half-block store overlap the psB matmuls;
            # relu(psB) runs on DVE. PSUM tiles reuse the chain{c} tags
            # so each bank frees as soon as its k=2 chain is cast.
            ob = None
            for ib in range(NI):
                psA = pb.tile([128, 512], f32, name=f"psA{ib}",
                              tag=f"chain{(2 * ib) % 8}")
                psB = pb.tile([128, 512], f32, name=f"psB{ib}",
                              tag=f"chain{(2 * ib + 1) % 8}")
                for tg, ps in ((0, psA), (1, psA), (2, psB), (3, psB)):
                    for k in range(K):
                        nc.tensor.matmul(
                            ps[:, ts(tg % 2, 4 * F_OUT)],
                            rts[k][tg][:, ts(ib, 128)],
                            th[:, ts(k, 4 * F_OUT)],
                            start=(k == 0),
                            stop=(k == K - 1),
                        )
                ob = ob_pool.tile([128, TO], bf16)
                nc.scalar.activation(ob[:, 0:512], psA[:],
                                     mybir.ActivationFunctionType.Relu)
                nc.sync.dma_start(out_d[ts(ib, 128), 0:512], ob[:, 0:512])
                if ib < NI - 1:
                    nc.vector.tensor_scalar_max(ob[:, 512:1024], psB[:], 0.0)
                    nc.scalar.dma_start(out_d[ts(ib, 128), 512:1024],
                                        ob[:, 512:1024])
                else:
                    # quarter-split the final block so the very last store is
                    # small and issues right after a short relu
                    nc.vector.tensor_scalar_max(ob[:, 512:768], psB[:, 0:256],
                                                0.0)
                    nc.scalar.dma_start(out_d[ts(ib, 128), 512:768],
                                        ob[:, 512:768])
                    nc.vector.tensor_scalar_max(ob[:, 768:1024], psB[:, 256:512],
                                                0.0)
                    nc.sync.dma_start(out_d[ts(ib, 128), 768:1024],
                                      ob[:, 768:1024])



    nc.compile()
    return nc


def _prep_inputs(x, att, cheb, Theta):
    import ml_dtypes

    bf16 = ml_dtypes.bfloat16
    # per-j int8 scales for cheb; s[j] and the att 1/255 fold into xT rows
    # (exact: RT contracts over j, x carries any per-j factor)
    s = np.abs(cheb).max(axis=(0, 1)) / 127.0          # [N] over (k, i)
    chebT = np.clip(np.rint(cheb / s[None, None, :]), -127, 127).astype(np.int8)
    chebT = np.ascontiguousarray(chebT.transpose(0, 2, 1)).reshape(K * N, N)
    xscale = (s * (1.0 / 255.0)).astype(np.float32)    # [N] per-j factor

    # zero-padded Theta: strip tt of the partition dim carries Theta[k]
    # only in the tt-th 64-col block of k's 256-col group
    thetaM = np.zeros((128, K * 4 * F_OUT), np.float32)
    for tt in range(4):
        for k in range(K):
            thetaM[tt * 32:(tt + 1) * 32,
                   k * 4 * F_OUT + tt * F_OUT:
                   k * 4 * F_OUT + (tt + 1) * F_OUT] = Theta[k]
    thetaM = thetaM.astype(bf16)

    in_maps = []
    for b in range(B):
        xb = np.ascontiguousarray(x[b].transpose(1, 0, 2)).reshape(N, TF)
        in_maps.append({
            "xT": (xb * xscale[:, None]).astype(bf16),
            "attT": np.rint(np.ascontiguousarray(att[b].T) * 255.0
                            ).astype(np.uint8),
            "chebT": chebT,
            "thetaM": thetaM,
        })
    return in_maps


def kernel(**inputs: np.ndarray) -> np.ndarray:
    global _LAST_RESULTS
    from concourse.bass_utils import run_bass_kernel_spmd

    x = np.asarray(inputs["x"], dtype=np.float32)
    att = np.asarray(inputs["spatial_attention"], dtype=np.float32)
    cheb = np.asarray(inputs["cheb"], dtype=np.float32)
    Theta = np.asarray(inputs["Theta"], dtype=np.float32)

    in_maps = _prep_inputs(x, att, cheb, Theta)
    nc = _build_bass()
    res = run_bass_kernel_spmd(nc, in_maps, core_ids=list(range(B)))
    _LAST_RESULTS = res

    out = np.stack(
        [r["out"].astype(np.float32).reshape(N, T, F_OUT).transpose(1, 0, 2)
         for r in res.results]
    )
    return out
